# revision 33
# baseline (speedup 1.0000x reference)
import numpy as np
import ml_dtypes

import concourse.tile as tile
from concourse import bacc, mybir
from concourse.bass_utils import run_bass_kernel_spmd

L, D_IN, D_HID, D_OUT, NTOT = 8, 256, 1024, 256, 32768
W = 512                       # max tokens per block (SBUF/PSUM tile width)
BLKS = [512] * 7 + [256, 256]  # small final blocks shorten the tail chain
COFF = [sum(BLKS[:i]) for i in range(len(BLKS) + 1)]
NBLK = len(BLKS)
P = COFF[-1]                  # 4096 padded tokens per core (one plane per core)
KI = D_IN // 128              # 2
MJ = D_HID // 128             # 8
MO = D_OUT // 128             # 2

F32 = mybir.dt.float32
BF16 = mybir.dt.bfloat16
AF = mybir.ActivationFunctionType
NPBF16 = ml_dtypes.bfloat16

WARMUP_MMS = 6                # dummy matmuls during initial DMA wait to warm the PE clock

PROFILE = False
LAST_RES = None
_nc_cache = None


def _build_nc():
    nc = bacc.Bacc()
    xp_d = nc.declare_dram_parameter("xp", [128, KI * P], BF16, isOutput=False)
    w1p_d = nc.declare_dram_parameter("w1p", [128, MJ * KI * 128], BF16, isOutput=False)
    w2p_d = nc.declare_dram_parameter("w2p", [128, MO * MJ * 128], BF16, isOutput=False)
    b1p_d = nc.declare_dram_parameter("b1p", [128, MJ], F32, isOutput=False)
    b2p_d = nc.declare_dram_parameter("b2p", [128, MO], F32, isOutput=False)
    outp_d = nc.declare_dram_parameter("outp", [128, MO * P], BF16, isOutput=True)

    with tile.TileContext(nc) as tc:
        with (
            tc.tile_pool(name="wpool", bufs=1) as wp,
            tc.tile_pool(name="xr", bufs=4) as xrp,
            tc.tile_pool(name="hr", bufs=2) as hrp,
            tc.tile_pool(name="outp", bufs=2) as outp,
            tc.tile_pool(name="ps1", bufs=6, space="PSUM") as ps1,
            tc.tile_pool(name="ps2", bufs=1, space="PSUM") as ps2,
        ):
            def x_load(ib):
                w = BLKS[ib]
                ts = []
                for k in range(KI):
                    r = xrp.tile([128, W], BF16, tag=f"x{k}")
                    o = KI * COFF[ib] + k * w
                    eng = nc.sync if k == 0 else nc.scalar
                    eng.dma_start(r[:, 0:w], xp_d[:, o:o + w])
                    ts.append(r)
                return ts

            # ---- critical-path DMA emission ----
            # sync ring:   w1j0, x0k0, w1j3, w1m(j4-5), w2, x-k0 stream,
            #              out-i1 stream, tail out-i0
            # scalar ring: x0k1, w1a(j1-2), b1, w1b(j6-7) (4 quick pushes, then
            #              the ACT table load + gelus own the sequencer),
            #              x-k1 stream, tail out-i1
            # gpsimd:      mid-stream out-i0 only (fire-and-forget SWDGE)
            w1j0 = wp.tile([128, KI * 128], BF16, tag="w1j0")
            nc.sync.dma_start(w1j0[:], w1p_d[:, 0:KI * 128])
            xb0_k1 = xrp.tile([128, W], BF16, tag="x1")
            nc.scalar.dma_start(xb0_k1[:], xp_d[:, W:2 * W])
            xb0_k0 = xrp.tile([128, W], BF16, tag="x0")
            nc.sync.dma_start(xb0_k0[:], xp_d[:, 0:W])
            w1a = wp.tile([128, 2 * KI * 128], BF16, tag="w1a")     # j1,j2
            nc.scalar.dma_start(w1a[:], w1p_d[:, KI * 128:3 * KI * 128])
            w1j3 = wp.tile([128, KI * 128], BF16, tag="w1j3")       # j3
            nc.sync.dma_start(w1j3[:], w1p_d[:, 3 * KI * 128:4 * KI * 128])
            w1m = wp.tile([128, 2 * KI * 128], BF16, tag="w1m")     # j4,j5
            nc.sync.dma_start(w1m[:], w1p_d[:, 4 * KI * 128:6 * KI * 128])
            b1_t = wp.tile([128, MJ], F32, tag="b1")
            nc.scalar.dma_start(b1_t[:], b1p_d[:])
            w1b = wp.tile([128, 2 * KI * 128], BF16, tag="w1b")     # j6,j7
            nc.scalar.dma_start(w1b[:], w1p_d[:, 6 * KI * 128:MJ * KI * 128])

            def w1_sl(j, k):
                if j == 0:
                    return w1j0[:, k * 128:(k + 1) * 128]
                if j <= 2:
                    o = ((j - 1) * KI + k) * 128
                    return w1a[:, o:o + 128]
                if j == 3:
                    return w1j3[:, k * 128:(k + 1) * 128]
                if j <= 5:
                    o = ((j - 4) * KI + k) * 128
                    return w1m[:, o:o + 128]
                o = ((j - 6) * KI + k) * 128
                return w1b[:, o:o + 128]
            w2r = [wp.tile([128, MJ * 128], BF16, tag=f"w2i{i}", name=f"w2i{i}")
                   for i in range(MO)]
            nc.sync.dma_start(w2r[0][:], w2p_d[:, 0:MJ * 128])
            nc.sync.dma_start(w2r[1][:], w2p_d[:, MJ * 128:2 * MJ * 128])

            # ---- PE warm-up: matmuls with no DMA dependency fill the init window ----
            if WARMUP_MMS:
                wdum = wp.tile([128, 128], BF16, tag="wdum")
                xdum = wp.tile([128, W], BF16, tag="xdum")
                nc.vector.memset(wdum[:], 0.0)
                nc.vector.memset(xdum[:], 0.0)
                # preload the Gelu activation table during the DMA wait window
                actd = wp.tile([128, 1], F32, tag="actd")
                nc.scalar.activation(actd[:], xdum[:, 0:1], AF.Gelu)
                psd = ps1.tile([128, W], F32, tag="h")
                for m in range(WARMUP_MMS):
                    nc.tensor.matmul(psd[:], wdum[:], xdum[:],
                                     start=(m == 0), stop=(m == WARMUP_MMS - 1))

            xcur = [xb0_k0, xb0_k1]
            xnxt = x_load(1)
            for ib in range(NBLK):
                w = BLKS[ib]
                xr = xcur
                xcur = xnxt
                xnxt = x_load(ib + 2) if ib + 2 < NBLK else None
                hr = []
                for j in range(MJ):
                    pt = ps1.tile([128, W], F32, tag="h")
                    for k in range(KI):
                        nc.tensor.matmul(pt[:, 0:w], w1_sl(j, k),
                                         xr[k][:, 0:w], start=(k == 0), stop=(k == KI - 1))
                    h = hrp.tile([128, W], BF16, tag=f"h{j}")
                    nc.scalar.activation(h[:, 0:w], pt[:, 0:w], AF.Gelu,
                                         bias=b1_t[:, j:j + 1])
                    hr.append(h)
                ot = outp.tile([128, MO * W], BF16, tag="o")
                # j-outer / i-inner: both output banks accumulate in parallel so
                # each gelu h[j] is consumed at 2 MMs per step (more slack for ACT)
                pt2s = [ps2.tile([128, W], F32, tag=f"o{i}", name=f"o{i}")
                        for i in range(MO)]
                for j in range(MJ):
                    for i in range(MO):
                        nc.tensor.matmul(pt2s[i][:, 0:w], w2r[i][:, j * 128:(j + 1) * 128],
                                         hr[j][:, 0:w], start=(j == 0), stop=(j == MJ - 1))
                # b2 is added host-side; drains only move PSUM -> SBUF (bf16).
                # Final block: drains split across vector and scalar engines,
                # pushes on both warm HWDGE rings, to shorten the tail chain.
                last = ib == NBLK - 1
                for i in range(MO):
                    od = outp_d[:, MO * COFF[ib] + i * w:MO * COFF[ib] + (i + 1) * w]
                    if last and i == 1:
                        nc.scalar.activation(ot[:, i * w:(i + 1) * w],
                                             pt2s[i][:, 0:w], AF.Copy)
                        nc.scalar.dma_start(od, ot[:, i * w:(i + 1) * w])
                    else:
                        nc.vector.tensor_scalar_add(ot[:, i * w:(i + 1) * w],
                                                    pt2s[i][:, 0:w], 0.0)
                        if last:
                            eng = nc.sync
                        else:
                            eng = nc.gpsimd if i == 0 else nc.sync
                        eng.dma_start(od, ot[:, i * w:(i + 1) * w])
    if not nc.is_finalized():
        nc.finalize()
    return nc


def _erf(z):
    # Abramowitz & Stegun 7.1.26, |err| <= 1.5e-7
    s = np.sign(z)
    z = np.abs(z)
    t = 1.0 / (1.0 + 0.3275911 * z)
    y = 1.0 - (((((1.061405429 * t - 1.453152027) * t) + 1.421413741) * t
                - 0.284496736) * t + 0.254829592) * t * np.exp(-z * z)
    return s * y


def _mlp_f64(xo, W1c, b1c, W2c, b2c):
    h = xo.astype(np.float64) @ W1c.T.astype(np.float64) + b1c.astype(np.float64)
    g = 0.5 * h * (1.0 + _erf(h / np.sqrt(2.0)))
    return (g @ W2c.T.astype(np.float64) + b2c.astype(np.float64)).astype(np.float32)


def kernel(x, W1, b1, W2, b2, plane_idx):
    global _nc_cache, LAST_RES
    x = np.ascontiguousarray(x, dtype=np.float32)
    W1 = np.asarray(W1, dtype=np.float32)
    b1 = np.asarray(b1, dtype=np.float32)
    W2 = np.asarray(W2, dtype=np.float32)
    b2 = np.asarray(b2, dtype=np.float32)
    plane_idx = np.asarray(plane_idx)

    xbf = x.astype(NPBF16)
    order = np.argsort(plane_idx, kind="stable")
    counts = np.bincount(plane_idx, minlength=L)
    starts = np.concatenate([[0], np.cumsum(counts)])

    in_maps = []
    idxs = []
    for c in range(L):
        idx = order[starts[c]:starts[c + 1]]
        idxs.append(idx)
        n = min(len(idx), P)
        xtb = np.zeros((D_IN, P), dtype=NPBF16)
        xtb[:, :n] = xbf[idx[:n]].T
        xp = np.empty((128, KI * P), dtype=NPBF16)
        for ib in range(NBLK):
            w = BLKS[ib]
            for k in range(KI):
                o = KI * COFF[ib] + k * w
                xp[:, o:o + w] = xtb[k * 128:(k + 1) * 128, COFF[ib]:COFF[ib] + w]
        w1p = np.ascontiguousarray(
            W1[c].T.reshape(KI, 128, MJ, 128).transpose(1, 2, 0, 3)
            .reshape(128, MJ * KI * 128).astype(NPBF16))
        w2p = np.ascontiguousarray(
            W2[c].T.reshape(MJ, 128, MO, 128).transpose(1, 2, 0, 3)
            .reshape(128, MO * MJ * 128).astype(NPBF16))
        in_maps.append({
            "xp": xp,
            "w1p": w1p,
            "w2p": w2p,
            "b1p": np.ascontiguousarray(b1[c].reshape(MJ, 128).T),
            "b2p": np.ascontiguousarray(b2[c].reshape(MO, 128).T),
        })

    if _nc_cache is None:
        _nc_cache = _build_nc()
    res = run_bass_kernel_spmd(_nc_cache, in_maps, list(range(L)), trace=PROFILE)
    LAST_RES = res

    out = np.empty((x.shape[0], D_OUT), dtype=np.float32)
    for c in range(L):
        idx = idxs[c]
        n = min(len(idx), P)
        op = np.asarray(res.results[c]["outp"]).astype(np.float32)
        outT = np.empty((D_OUT, P), dtype=np.float32)
        for ib in range(NBLK):
            w = BLKS[ib]
            for i in range(MO):
                o = MO * COFF[ib] + i * w
                outT[i * 128:(i + 1) * 128, COFF[ib]:COFF[ib] + w] = op[:, o:o + w]
        out[idx[:n]] = outT[:, :n].T + b2[c][None, :]
        if len(idx) > n:
            out[idx[n:]] = _mlp_f64(x[idx[n:]], W1[c], b1[c], W2[c], b2[c])
    return out
